# revision 54
# baseline (speedup 1.0000x reference)
"""Trainium2 Bass kernel for nn_MultiHeadSelfAttentionModule_6193342840934.

Reference math (per batch row b of x[B,S,D]):
    xn  = LayerNorm(x) * ln_g + ln_b
    Q/K/V = xn @ w{q,k,v} + b{q,k,v}   (heads H=16, dk=64)
    scores = Q K^T / sqrt(dk) + rel_bias[h]          (S=32)
    out = x + softmax(scores) @ V @ wo + bo

Distribution: pure data-parallel over the batch dim, 2048/8 = 256 batches
(8192 tokens) per NeuronCore. Weights are replicated to every core.

Per-core kernel layout strategy:
  - tokens processed in 512-token super-tiles (16 per core); x uploaded
    bf16 and y returned bf16 (host casts) — halves both HBM streams, the
    bf16 noise is far below the fp8 noise already in the matmul path.
  - all four D x D projections run as fp8e4 DoubleRow matmuls (both
    operands fp8, 256-deep contraction per matmul, 2x PE throughput;
    weights quantized host-side, xn/ctx quantized at evacuation).
    End-to-end rel err ~7.8e-3 on HW.
  - LayerNorm token-major via bn_stats; ln_g/ln_b folded into the weights
    host-side (exact): wq' = ln_g*wq, bq' = bq + ln_b@wq, etc. rsig
    computed as exp(-0.5*ln(var+eps)) so the ONLY ACT functions used are
    {identity, copy, exp, ln} — one activation table, zero table-swap
    stalls. The LN apply itself runs on DVE (single-src tensor_scalar
    with per-partition AP scale/bias), keeping the ACT engine off the
    projection-phase critical path.
  - xn transposed to d-major via regular fp8 matmuls against an fp8
    identity (PE transpose-mode is slower per 128x128 on HW).
  - scores computed TRANSPOSED (scoresT[k,q] = K'^T Q) so that softmax's
    denominator can be computed with a block-diagonal-ones matmul over the
    partition dim and the attention application needs no transpose of the
    attention matrix. 1/sqrt(dk) folded into K evacuation, rel_bias added
    via an identity-matmul accumulated into the same PSUM region.
  - IMPORTANT HW CONSTRAINT (found empirically on this runtime): consecutive
    matmuls whose operands sit at different base partitions (different PE
    row groups) crash the device. Every matmul therefore uses the full
    128-partition contraction; sub-128 contractions are expressed by
    zero-padding one operand (exact in fp: 0*x = 0), and only the PSUM
    column position varies via tile_position.
  - Q lands in a per-head-pair BLOCK-DIAGONAL layout qbd (built by
    SBUF->SBUF DMA from the dense evac; DMA engines otherwise idle):
    heads 2c/2c+1 occupy partition halves of slot c with zeros elsewhere,
    so ONE score matmul per (head-pair, batch) with the dense K tile as
    lhsT computes both heads over the full 128-partition contraction —
    half the score-MM count and half the LDWEIGHTS columns of a per-head
    zero-padded layout. qbd/at_bd zero regions are persistent, memset once.
  - softmax without max-subtraction (scores are O(10) here; exp is safe in
    fp32); denominators via ones-matmul; 1/denom via exp(-ln(x)) on ACT,
    emitted in bf16 so the at_bd normalization multiplies run in DVE
    2x_1P mode; the normalization writes the per-batch block-diagonal
    at_bd tiles directly (no separate attn tile or copies).
  - phase-pipelined schedule: scores+exp for all 4 sub-tiles stream first;
    the next super-tile's LN/transposes + V projection (V only needs the
    current sub-tile's xnT columns as the stationary operand, so it runs
    per sub-tile right after that sub-tile's transposes) occupy the PE
    while this super-tile's softmax-normalization rides between those
    matmul groups; Q/K projections follow with the full xnT; pass_out
    (AV + output projection + residual) runs last with everything ready.
  - ctx computed d-major, 16 AV matmuls split across two single-bank PSUM
    tiles in a 2-deep ring (next sub-tile's AV never waits on this one's
    evacuation), output projection token-major, residual added by the
    PSUM-evacuating tensor_add.
"""

import os

import numpy as np
import ml_dtypes

# timing-attribution ablations (wrong numerics, same dataflow shape):
#   BASS_MHSA_ABLATE=scores4   only every 4th head's score matmuls
#   BASS_MHSA_ABLATE=noctx     1 of the 16 per-head AV matmuls
#   BASS_MHSA_ABLATE=bf16proj  projections as bf16 pairs, no fp8 DoubleRow
#   BASS_MHSA_ABLATE=tmode     PE transpose-mode instead of normal fp8 matmuls
ABLATE = os.environ.get("BASS_MHSA_ABLATE", "")

import concourse.bass as bass
import concourse.tile as tile
import concourse.mybir as mybir
from concourse.alu_op_type import AluOpType
from concourse.vector_clock import ScopedClock

dt = mybir.dt
AF = mybir.ActivationFunctionType
PM = mybir.MatmulPerfMode

B, S, D, H = 2048, 32, 1024, 16
DK = D // H          # 64
EPS = 1e-5
N_CORES = 8
BPC = B // N_CORES   # 256 batches per core
TPC = BPC * S        # 8192 tokens per core
ST = 512             # tokens per super-tile
NSUB = ST // 128     # 4 sub-tiles of 128 tokens
NSUP = TPC // ST     # 16 super-tiles
NCH = D // 128       # 8 d-chunks

BF16 = ml_dtypes.bfloat16
F8 = ml_dtypes.float8_e4m3


class SplitDrainTileContext(tile.TileContext):
    """This container's walrus build rejects >1 sync-wait on a Drain
    instruction; split the tail drain's waits across standalone NOPs."""

    def _drain_and_barrier(self, tick_clock, wait_clock):
        drain_inst = self.nc.sync.drain()
        wait_clock.add_sem_waits(
            drain_inst.ins, ScopedClock({None: tick_clock.global_clock})
        )
        si = drain_inst.ins.sync_info
        waits = list(si.on_wait or []) if si is not None else []
        if len(waits) > 1:
            drain_inst.ins.sync_info.on_wait = waits[:1]
            for w in waits[1:]:
                nop = self.nc.sync.nop(hint="drain_split_wait", nofuse=True)
                nop.ins.sync_info = mybir.SyncInfo(on_wait=[w], on_update=[])
        self.nc.all_engine_barrier()
        assert self.sems is not None
        popped = self.nc._tile_sem_poison_stack.pop()
        assert popped is self._sem_poison
        self.nc.clear_and_free_semaphores(list(self.sems.allocated().values()))
        self.nc.all_engine_barrier()


def _split_excess_waits(nc: bass.Bass):
    """This container's walrus accepts at most 1 sync-wait per instruction
    (2 for EventSemaphore), but this tile version assigns up to 4. Move
    excess waits onto injected same-engine NoOps right before the
    instruction — engine streams are in-order, so this is equivalent."""
    for f in nc.m.functions:
        for bb in f.blocks:
            insts = list(bb.instructions)
            out = []
            changed = False
            for inst in insts:
                si = inst.sync_info
                cap = 2 if inst.opcode == "EventSemaphore" else 1
                waits = list(si.on_wait) if si is not None and si.on_wait else []
                if len(waits) > cap:
                    changed = True
                    for w in waits[cap:]:
                        nop = mybir.InstNoOp(
                            name=nc.get_next_instruction_name(),
                            engine=inst.engine,
                            sync_info=mybir.SyncInfo(on_wait=[w], on_update=[]),
                            bass_nofuse=True,
                        )
                        out.append(nop)
                    inst.sync_info = mybir.SyncInfo(
                        on_wait=waits[:cap], on_update=list(si.on_update or [])
                    )
                out.append(inst)
            if changed:
                bb.instructions = out


def build_nc(repeat: int = 1, split_waits: bool = True) -> bass.Bass:
    """Build the per-core Bass module. repeat>1 wraps the body in a hardware
    loop (used only for benchmarking slope timing). split_waits applies the
    walrus 1-wait-per-instruction workaround (disable for CoreSim runs)."""
    nc = bass.Bass("TRN2", target_bir_lowering=False, debug=False, num_devices=1)

    f32 = dt.float32
    bf16 = dt.bfloat16
    f8 = dt.float8e4
    wddt = bf16 if ABLATE == "bf16proj" else f8

    # x is uploaded as bf16 (host-side cast): halves the input HBM traffic;
    # bn_stats/LN-apply/residual-add all tolerate the 0.4% bf16 noise (the
    # attention path quantizes to fp8 anyway and the residual error is well
    # inside the rel-err budget).
    x_d = nc.dram_tensor("x", [TPC, D], bf16, kind="ExternalInput").ap()
    # y in bf16: halves the output HBM traffic; the host upcasts to f32.
    y_d = nc.dram_tensor("y", [TPC, D], bf16, kind="ExternalOutput").ap()
    # weights are pre-laid-out host-side so every matmul's weight slice is
    # CONTIGUOUS per partition (the PE's weight-load path streams strided
    # APs slower): wq/wk as [p, c_out, cp, two, 128] (lhsT blocks), wv/wo
    # as [p, cp, half, two, 512] (rhs blocks).
    wq_d = nc.dram_tensor(
        "wqs", [128, NCH, NCH // 2, 2, 128], wddt, kind="ExternalInput").ap()
    wk_d = nc.dram_tensor(
        "wks", [128, NCH, NCH // 2, 2, 128], wddt, kind="ExternalInput").ap()
    wv_d = nc.dram_tensor(
        "wvs", [128, NCH // 2, 2, 2, 512], wddt, kind="ExternalInput").ap()
    wo_d = nc.dram_tensor(
        "wos", [128, NCH // 2, 2, 2, 512], wddt, kind="ExternalInput").ap()
    bq_d = nc.dram_tensor("bq_eff", [128, NCH], f32, kind="ExternalInput").ap()
    bk_d = nc.dram_tensor("bk_eff", [128, NCH], f32, kind="ExternalInput").ap()
    # relpad[j, h*32+q] = rel_bias[h, q, j] for j<32, 0 for j>=32
    rel_d = nc.dram_tensor("relpad", [128, H * 32], bf16, kind="ExternalInput").ap()
    id_d = nc.dram_tensor("ident", [128, 128], bf16, kind="ExternalInput").ap()
    id8_d = nc.dram_tensor("ident8", [128, 128], f8, kind="ExternalInput").ap()
    # id4pad[j, p] = (j == p % 32) for j<32, 0 for j>=32
    id4_d = nc.dram_tensor("id4pad", [128, 128], bf16, kind="ExternalInput").ap()
    # exprel[(b,k), (h,q)] = exp(rel_bias[h, q, k])  (tiled over the 4 blocks)
    exprel_d = nc.dram_tensor("exprel", [128, H * 32], bf16, kind="ExternalInput").ap()
    # bdones[(b,k), (b',m)] = (b == b')  (32-block diagonal of ones)
    bdon_d = nc.dram_tensor("bdones", [128, 128], bf16, kind="ExternalInput").ap()

    with SplitDrainTileContext(nc) as tc:
        with (
            tc.tile_pool(name="consts", bufs=1) as consts,
            tc.tile_pool(name="xin", bufs=(6 if ABLATE == "bf16proj" else 16)) as xin_pool,
            tc.tile_pool(name="small", bufs=8) as small,
            tc.tile_pool(name="xn0", bufs=2) as xn0_pool,
            tc.tile_pool(name="xnT", bufs=2) as xnT_pool,
            tc.tile_pool(name="qk", bufs=2) as qk_pool,
            tc.tile_pool(name="vsb", bufs=2) as v_pool,
            tc.tile_pool(name="attn", bufs=2) as attn_pool,
            tc.tile_pool(name="atu", bufs=4) as atu_pool,
            tc.tile_pool(name="ctx", bufs=2) as ctx_pool,
            tc.tile_pool(name="osb", bufs=(1 if ABLATE == "bf16proj" else 2)) as out_pool,
            tc.tile_pool(name="ps_proj", bufs=2, space="PSUM") as ps_proj,
            tc.tile_pool(name="ps_attn", bufs=2, space="PSUM") as ps_attn,
            tc.tile_pool(name="ps_ctx", bufs=2, space="PSUM") as ps_ctx,
            tc.tile_pool(name="ps_xp", bufs=2, space="PSUM") as ps_xp,
        ):
            # -- resident constants -------------------------------------------
            wdt = bf16 if ABLATE == "bf16proj" else f8
            wq_s = consts.tile([128, NCH, NCH // 2, 2, 128], wdt)
            wk_s = consts.tile([128, NCH, NCH // 2, 2, 128], wdt)
            wv_s = consts.tile([128, NCH // 2, 2, 2, 512], wdt)
            wo_s = consts.tile([128, NCH // 2, 2, 2, 512], wdt)
            for wsb, wd in ((wq_s, wq_d), (wk_s, wk_d), (wv_s, wv_d), (wo_s, wo_d)):
                nc.sync.dma_start(wsb, wd)
            bq_s = consts.tile([128, NCH], f32)
            nc.sync.dma_start(bq_s, bq_d)
            bk_s = consts.tile([128, NCH], f32)
            nc.sync.dma_start(bk_s, bk_d)
            rel_s = consts.tile([128, H * 32], bf16)
            nc.sync.dma_start(rel_s, rel_d)
            id_s = consts.tile([128, 128], bf16)
            nc.sync.dma_start(id_s, id_d)
            id8_s = consts.tile([128, 128], f8)
            nc.sync.dma_start(id8_s, id8_d)
            id4_s = consts.tile([128, 128], bf16)
            nc.sync.dma_start(id4_s, id4_d)
            exprel_s = consts.tile([128, H * 32], bf16)
            nc.sync.dma_start(exprel_s, exprel_d)
            bdon_s = consts.tile([128, 128], bf16)
            nc.sync.dma_start(bdon_s, bdon_d)
            eps_s = consts.tile([128, 1], f32)
            nc.vector.memset(eps_s, EPS)

            # persistent zero-padded tiles (double-buffered by hand): the
            # zero regions are written once here and never touched again —
            # evacuations only write the valid blocks, so the per-super-tile
            # gpsimd memsets of the baseline are hoisted out of the loop.
            # qbd holds Q in a per-head-pair BLOCK-DIAGONAL layout:
            # qbd[0:64, c, 0, :] = Q of head 2c, qbd[64:128, c, 1, :] = Q of
            # head 2c+1, zeros elsewhere — so ONE score matmul per (pair,
            # batch) with dense ks as lhsT computes both heads' scores over
            # the full 128-partition contraction (zeros kill the cross-head
            # terms). Halves both the score-MM count and the LDWEIGHTS
            # column traffic vs the per-head zero-padded K layout.
            qbds = []
            for i in range(1 if ABLATE == "bf16proj" else 2):
                qb = consts.tile([128, NCH, 2, ST], bf16, tag=f"qbdp{i}")
                nc.gpsimd.memset(qb, 0.0)
                qbds.append(qb)
            # one at_bd per sub-tile: the norm pass (interleaved into the
            # next supertile's projections) writes all four before pass_out
            # reads any — a 2-deep ring would overwrite live data.
            at_bds = []
            for i in range(NSUB):
                ab = consts.tile([128, H, 128], bf16, tag=f"atbdp{i}")
                nc.gpsimd.memset(ab, 0.0)
                at_bds.append(ab)

            # per-super-tile prelude state (xts list + mv4 + xnT tile).
            # prelude_load(sup, s) DMAs one 128-token tile and computes its
            # bn stats into a shared [128, 4, 2] tile; prelude_finish(sup)
            # batches the LN scalar chain across all 4 tiles (4x fewer tiny
            # ops), applies LN to fp8, and transposes via regular fp8
            # matmuls against an fp8 identity (the PE transpose-mode path
            # is ~2-3x slower per 128x128 on HW than a plain N=128 matmul).
            state: dict = {}

            def prelude_load(sup: int, s: int):
                t0 = sup * ST
                if s == 0:
                    xnT = xnT_pool.tile(
                        [128, NCH, ST], bf16 if ABLATE == "bf16proj" else f8, tag="xnT"
                    )
                    mv4 = small.tile([128, 4, 2], f32, tag="mv4")
                    state[sup] = ([], xnT, mv4)
                xts, xnT, mv4 = state[sup]
                row = t0 + s * 128
                xt = xin_pool.tile([128, D], bf16, tag="x")
                nc.sync.dma_start(xt, x_d[row : row + 128, :])
                xts.append(xt)
                st6 = small.tile([128, 2, 6], f32, tag="st6")
                nc.vector.bn_stats(st6[:, 0, :], xt[:, 0:512])
                nc.vector.bn_stats(st6[:, 1, :], xt[:, 512:1024])
                nc.vector.bn_aggr(mv4[:, s, :], st6)

            lnstate: dict = {}

            def prelude_ln(sup: int):
                """LN scalar chain + sub-tile 0's LN apply for supertile sup,
                hoisted into the PREVIOUS fused pass: the ACT ln/exp + DVE
                multiply chain completes while the PE streams that pass's
                matmuls, so prelude_finish(sup)'s first transpose issues
                immediately instead of waiting ~2us on the chain.
                rsig = exp(-0.5*ln(var+eps)): keeps every ACT function used
                by this kernel (identity/copy/exp/ln) inside ONE activation
                table — an AF.Sqrt here would force two table swaps per
                super-tile right in the softmax chain's way."""
                xts, _, mv4 = state[sup]
                lnv4 = small.tile([128, 4], f32, tag="lnv4")
                nc.scalar.activation(lnv4, mv4[:, :, 1], AF.Ln, bias=eps_s[:])
                rsig4 = small.tile([128, 4], f32, tag="rsig4")
                nc.scalar.activation(rsig4, lnv4, AF.Exp, scale=-0.5)
                nmu4 = small.tile([128, 4], f32, tag="nmu4")
                nc.vector.tensor_mul(nmu4, mv4[:, :, 0], rsig4)
                nmr4 = small.tile([128, 4], f32, tag="nmr4")
                nc.vector.tensor_scalar_mul(nmr4, nmu4, -1.0)
                xn0 = xn0_pool.tile([128, D], f8, tag="xn0")
                nc.vector.tensor_scalar(
                    xn0, xts[0], rsig4[:, 0:1], nmr4[:, 0:1],
                    AluOpType.mult, AluOpType.add,
                )
                lnstate[sup] = (rsig4, nmr4, xn0)

            def prelude_finish(sup: int):
                """LN scalar chain + per-subtile: LN-apply, transposes, V
                projection, and (interleaved) norm_sup's softmax
                normalization. V only needs THIS subtile's xnT columns (it is
                the matmul's stationary side), so it runs right after the
                subtile's transposes — the PE gets dense matmul work per
                subtile instead of idling until the whole xnT tile is built,
                and the norm chain rides between those matmul groups."""
                xts, xnT, mv4 = state[sup]
                rsig4, nmr4, xn0_first = lnstate.pop(sup)

                def ln_transpose(s: int):
                    if s == 0:
                        # LN apply for sub-tile 0 was hoisted into the
                        # previous fused pass (prelude_ln) — the first
                        # transposes start with zero chain latency.
                        xn0 = xn0_first
                    else:
                        xn0 = xn0_pool.tile([128, D], f8, tag="xn0")
                        # LN apply on DVE (single-src tensor_scalar with
                        # per-partition AP scalars) — off the ACT engine,
                        # which gates the projection phase start.
                        nc.vector.tensor_scalar(
                            xn0, xts[s], rsig4[:, s : s + 1], nmr4[:, s : s + 1],
                            AluOpType.mult, AluOpType.add,
                        )
                    for half in range(2):
                        if ABLATE == "tmode":
                            xp = ps_xp.tile([128, NCH // 2, 128], f8, tag="xp")
                            for c in range(NCH // 2):
                                cc = half * (NCH // 2) + c
                                nc.tensor.transpose(
                                    xp[:, c, :], xn0[:, cc * 128 : (cc + 1) * 128], id8_s
                                )
                        else:
                            xp = ps_xp.tile([128, NCH // 2, 128], f32, tag="xp")
                            for c in range(NCH // 2):
                                cc = half * (NCH // 2) + c
                                nc.tensor.matmul(
                                    xp[:, c, :],
                                    lhsT=xn0[:, cc * 128 : (cc + 1) * 128],
                                    rhs=id8_s,
                                    start=True,
                                    stop=True,
                                    skip_group_check=True,
                                )
                        # alternate the PSUM->SBUF evacuation between DVE and
                        # ACT: DVE is the busiest engine, ACT has slack.
                        evac = nc.vector.tensor_copy if half == 0 else (
                            lambda o, i: nc.scalar.activation(o, i, AF.Copy)
                        )
                        evac(
                            xnT[:, half * (NCH // 2) : (half + 1) * (NCH // 2),
                                s * 128 : (s + 1) * 128],
                            xp,
                        )

                for s in range(NSUB):
                    ln_transpose(s)

            def contract_w(ps, w_r, c, xnT):
                """Q/K d-contraction: weight-stationary, lhsT = contiguous
                [p, 2, 128] blocks of the re-laid weight, rhs = xnT chunk
                pairs (contiguous). fp8 DoubleRow, or bf16 under ablation."""
                if ABLATE == "bf16proj":
                    for ci in range(NCH):
                        nc.tensor.matmul(
                            ps,
                            lhsT=w_r[:, c, ci // 2, ci % 2, :],
                            rhs=xnT[:, ci, :],
                            start=(ci == 0),
                            stop=(ci == NCH - 1),
                        )
                else:
                    ncp = NCH // 4 if ABLATE == "projhalf" else NCH // 2
                    for cp in range(ncp):
                        nc.tensor.matmul(
                            ps,
                            lhsT=w_r[:, c, cp, :, :],
                            rhs=xnT[:, 2 * cp : 2 * cp + 2, :],
                            start=(cp == 0),
                            stop=(cp == ncp - 1),
                            perf_mode=PM.DoubleRow,
                        )

            def contract_x(ps, lhs_tile, lhs_cols, w_r, half):
                """V/O d-contraction: activation-stationary, rhs = contiguous
                [p, 2, 512] blocks of the re-laid weight."""
                if ABLATE == "bf16proj":
                    for ci in range(NCH):
                        nc.tensor.matmul(
                            ps,
                            lhsT=lhs_tile[:, ci, lhs_cols],
                            rhs=w_r[:, ci // 2, half, ci % 2, :],
                            start=(ci == 0),
                            stop=(ci == NCH - 1),
                        )
                else:
                    ncp = NCH // 4 if ABLATE == "projhalf" else NCH // 2
                    for cp in range(ncp):
                        nc.tensor.matmul(
                            ps,
                            lhsT=lhs_tile[:, 2 * cp : 2 * cp + 2, lhs_cols],
                            rhs=w_r[:, cp, half, :, :],
                            start=(cp == 0),
                            stop=(cp == ncp - 1),
                            perf_mode=PM.DoubleRow,
                        )

            # per-supertile attention state carried between the pass functions
            astate: dict = {}
            vstate: dict = {}

            def emit_norm(sup: int, s: int):
                """Softmax denominator + reciprocal + block-diagonalization
                for one sub-tile. Emitted interleaved between projection
                chunk groups so the PE always has matmul work while the
                ACT ln/exp + DVE multiply chain completes."""
                at_us = astate[sup][2]
                # per-batch-block softmax denominators, replicated across
                # each 32-row block by the block-diagonal ones matmul
                dn = ps_attn.tile([128, H * 32], f32, tag="attn")
                nc.tensor.matmul(dn, lhsT=bdon_s, rhs=at_us[s], start=True, stop=True)
                # 1/denom via exp(-ln(x)) on ACT: this walrus build rejects
                # the custom-DVE fast-reciprocal ISA op, and the native DVE
                # reciprocal is ~8 cyc/elem. LUT rel-err ~1e-4 is fine at
                # bf16 noise levels.
                lnd = attn_pool.tile([128, H * 32], f32, tag="lnd")
                nc.scalar.activation(lnd, dn, AF.Ln)
                # rc in bf16: the at_bd normalization multiplies below then
                # run in DVE 2x_1P mode (both operands 16-bit) — ~2x faster.
                rc = attn_pool.tile([128, H * 32], bf16, tag="rc")
                nc.scalar.activation(rc, lnd, AF.Exp, scale=-1.0)
                # normalize (at_u * rc) fused directly into the per-batch
                # block-diagonal writes: at_bd[(b,k), h, (b,q)] nonzero
                # only for matching b, so AV can contract over the full
                # 128 token partitions. Zero regions are persistent.
                at_bd = at_bds[s % len(at_bds)]
                atuv = at_us[s].rearrange("p (h q) -> p h q", h=H)
                rcv = rc.rearrange("p (h q) -> p h q", h=H)
                for b in range(4):
                    blk = slice(b * 32, (b + 1) * 32)
                    nc.vector.tensor_mul(
                        at_bd[blk, :, blk], atuv[blk, :, :], rcv[blk, :, :]
                    )

            def proj(sup: int):
                """Q/K projections for supertile sup (fp8 DoubleRow); V runs
                inside prelude_finish."""
                _, xnT, _ = state[sup]
                qs = qk_pool.tile([128, NCH, ST], bf16, tag="q")
                ks = qk_pool.tile([128, NCH, ST], bf16, tag="k")
                qbd = qbds[sup % len(qbds)]
                astate[sup] = (qbd, ks)
                for c in range(NCH):
                    ps = ps_proj.tile([128, 512], f32, tag="proj")
                    contract_w(ps, wq_s, c, xnT)
                    # full-partition evac to dense qs; the block-diagonal qbd
                    # layout is built by SBUF->SBUF DMA (DMA engines are
                    # otherwise mostly idle), staged per 4 chunks to hide the
                    # copy latency behind the remaining projections.
                    nc.scalar.activation(
                        qs[:, c, :], ps, AF.Identity, bias=bq_s[:, c : c + 1], scale=1.0
                    )
                    if c == 3 or c == 7:
                        c0 = c - 3
                        nc.sync.dma_start(
                            qbd[0:64, c0 : c + 1, 0, :], qs[0:64, c0 : c + 1, :]
                        )
                        nc.sync.dma_start(
                            qbd[64:128, c0 : c + 1, 1, :], qs[64:128, c0 : c + 1, :]
                        )
                for c in range(NCH):
                    ps = ps_proj.tile([128, 512], f32, tag="proj")
                    contract_w(ps, wk_s, c, xnT)
                    nc.scalar.activation(
                        ks[:, c, :], ps, AF.Identity,
                        bias=bk_s[:, c : c + 1], scale=0.125,
                    )

            def scores_subtile(sup: int, s: int):
                """Scores + exp for ONE sub-tile of supertile sup.
                scoresT[(b,k), (h,q)] = K'^T Q + rel_biasT  (PSUM bank).
                NB: skip_group_check — the sim's coarse PSUM zero-region
                bookkeeping can't express "one full-region start, many
                sub-block accumulates"; on HW this is per-element
                has_written and PE executes in program order."""
                st = astate[sup]
                if len(st) == 2:
                    astate[sup] = st + ([],)
                qbd, ks, at_us = astate[sup]
                sc = ps_attn.tile([128, H * 32], f32, tag="attn")
                nc.tensor.matmul(
                    sc, lhsT=id4_s, rhs=rel_s, start=True, stop=False,
                    skip_group_check=True,
                )
                # one matmul per (head-pair, batch): lhsT = dense ks
                # (heads 2c/2c+1 stacked on the partition dim), rhs = the
                # block-diagonal qbd slice [128, 2, 32] -> 64 output cols
                # (j2, q) landing exactly on the (h, q) column layout.
                sc_pairs = (
                    list(range(0, NCH, 4)) if ABLATE == "scores4" else list(range(NCH))
                )
                for ci, c in enumerate(sc_pairs):
                    for b in range(4):
                        tok = slice(s * 128 + b * 32, s * 128 + (b + 1) * 32)
                        nc.tensor.matmul(
                            sc[b * 32 : (b + 1) * 32, c * 64 : (c + 1) * 64],
                            lhsT=ks[:, c, tok],
                            rhs=qbd[:, c, :, tok],
                            start=False,
                            stop=(ci == len(sc_pairs) - 1),
                            tile_position=(0, b * 32),
                            skip_group_check=True,
                        )
                at_u = atu_pool.tile([128, H * 32], bf16, tag="atu")
                nc.scalar.activation(at_u, sc, AF.Exp)
                at_us.append(at_u)

            def vproj(sup: int, s: int):
                """V projection for one sub-tile, emitted inside the
                chain-paced fused pass (PE slack) instead of the PE-paced
                prelude — V is only consumed by the NEXT pass's AV matmuls."""
                if s == 0:
                    vsn = v_pool.tile([128, NSUB, D], bf16, tag="v")
                    vstate[sup] = vsn
                vs = vstate[sup]
                _, xnT, _ = state[sup]
                for half in range(2):
                    ps = ps_proj.tile([128, 512], f32, tag="proj")
                    contract_x(ps, xnT, slice(s * 128, (s + 1) * 128), wv_s, half)
                    if half == 0:
                        nc.scalar.activation(vs[:, s, 0:512], ps, AF.Copy)
                    else:
                        nc.vector.tensor_copy(vs[:, s, 512:1024], ps)

            def pass_out(sup: int, nxt: int | None = None):
                """AV matmuls + output projection + residual, per sub-tile,
                with the NEXT supertile's score matmuls, softmax
                normalization, and prefetch loads interleaved: this pass is
                chain-paced with PE slack, while prelude+projection is
                PE-paced — so ALL movable work (scores, the dn matmul, the
                ACT ln/exp + DVE at_bd chain) rides here. The at_bd[s] write
                of norm(nxt) lands right after this pass's own AV(s) read of
                the same shared tile."""
                t0 = sup * ST
                xts = state[sup][0]
                vs = vstate.pop(sup)
                for s in range(NSUB):
                    # ctxT[(h,dv), t] d-major: one matmul per head over all 4
                    # batches at once (cross-batch terms killed by at_bd
                    # zeros). Heads are split across TWO single-bank PSUM
                    # tiles in a 2-deep ring so the next sub-tile's AV
                    # matmuls never stall on this sub-tile's evacuation.
                    at_bd = at_bds[s % len(at_bds)]
                    ctxT = ctx_pool.tile(
                        [128, NCH, 128], bf16 if ABLATE == "bf16proj" else f8, tag="ctxT"
                    )
                    nheads = 1 if ABLATE == "noctx" else H
                    for g in range(2):
                        cps = ps_ctx.tile([128, 4, 128], f32, tag="ctx")
                        for h in range(g * 8, min((g + 1) * 8, nheads)):
                            pb = (h % 2) * 64
                            nc.tensor.matmul(
                                cps[pb : pb + 64, (h // 2) % 4, :],
                                lhsT=vs[:, s, h * 64 : (h + 1) * 64],
                                rhs=at_bd[:, h, :],
                                start=True,
                                stop=True,
                                tile_position=(0, pb),
                                skip_group_check=True,
                            )
                        if nheads > g * 8 or g == 0:
                            nc.scalar.activation(
                                ctxT[:, g * 4 : (g + 1) * 4, :], cps, AF.Copy
                            )

                    # next supertile's score + V-projection matmuls ride
                    # here: the PE has independent work while ctxT evacuates.
                    if nxt is not None:
                        scores_subtile(nxt, s)
                        vproj(nxt, s)
                        # hoist supertile nxt+1's LN chain under this pass's
                        # matmul stream (see prelude_ln).
                        if s == 1 and nxt + 1 < NSUP:
                            prelude_ln(nxt + 1)

                    # out = x + ctx @ wo   (token-major, fp8 DoubleRow)
                    outsb = out_pool.tile([128, D], bf16, tag="osb")
                    for half in range(2):
                        ps = ps_proj.tile([128, 512], f32, tag="proj")
                        contract_x(ps, ctxT, slice(None), wo_s, half)
                        nc.vector.tensor_add(
                            outsb[:, half * 512 : (half + 1) * 512],
                            xts[s][:, half * 512 : (half + 1) * 512],
                            ps,
                        )
                    row = t0 + s * 128
                    nc.sync.dma_start(y_d[row : row + 128, :], outsb)
                    if nxt is not None:
                        emit_norm(nxt, s)
                    # x loads + bn stats for supertile nxt+2 are emitted
                    # AFTER this sub-tile's residual add (the load's xin buf
                    # is freed by an earlier add, keeping the DVE queue
                    # deadlock-free) — still a full pipeline iteration ahead
                    # of prelude_finish(nxt+2)'s LN chain.
                    if nxt is not None and nxt + 2 < NSUP:
                        prelude_load(nxt + 2, s)
                del astate[sup]
                del state[sup]

            def run_all():
                # pipeline: while the PE chews supertile sup+1's projections,
                # the ACT/DVE softmax chain of supertile sup completes, so
                # pass_norm/pass_out never stall the PE stream for long.
                for s in range(NSUB):
                    prelude_load(0, s)
                prelude_ln(0)
                prelude_finish(0)
                for s in range(NSUB):
                    prelude_load(1, s)
                prelude_ln(1)
                proj(0)
                # first supertile's scores run standalone (no pass_out to
                # fuse into); supertile 2's loads ride along.
                for s in range(NSUB):
                    prelude_load(2, s)
                    scores_subtile(0, s)
                    vproj(0, s)
                    emit_norm(0, s)
                for sup in range(NSUP):
                    if sup + 1 < NSUP:
                        prelude_finish(sup + 1)
                        proj(sup + 1)
                        pass_out(sup, nxt=sup + 1)
                    else:
                        pass_out(sup, nxt=None)

            if repeat > 1:
                with tc.For_i(0, repeat, 1):
                    run_all()
            else:
                run_all()

    if split_waits:
        _split_excess_waits(nc)
    return nc


def _host_constants(ln_g, ln_b, wq, bq, wk, bk, wv, bv, wo, bo, rel_bias):
    """Exact host-side weight transforms (fold LN affine + 1/sqrt(dk))."""
    f32 = np.float32
    g = ln_g.astype(f32)
    b = ln_b.astype(f32)
    wq = wq.astype(f32)
    wk = wk.astype(f32)
    wv = wv.astype(f32)
    wo = wo.astype(f32)
    wdt = BF16 if ABLATE == "bf16proj" else F8
    # re-layout so every on-device weight slice is contiguous per partition:
    #   wq/wk: [p, c_out, cp, two, 128]  (lhsT blocks, d_in=(cp,two,p))
    #   wv/wo: [p, cp, half, two, 512]   (rhs blocks)
    wqs = np.ascontiguousarray(
        (g[:, None] * wq).astype(wdt).reshape(4, 2, 128, 8, 128)
        .transpose(2, 3, 0, 1, 4)
    )
    wks = np.ascontiguousarray(
        (g[:, None] * wk).astype(wdt).reshape(4, 2, 128, 8, 128)
        .transpose(2, 3, 0, 1, 4)
    )
    wvs = np.ascontiguousarray(
        (g[:, None] * wv).astype(wdt).reshape(4, 2, 128, 2, 512)
        .transpose(2, 0, 3, 1, 4)
    )
    wos = np.ascontiguousarray(
        wo.astype(wdt).reshape(4, 2, 128, 2, 512).transpose(2, 0, 3, 1, 4)
    )
    bq_eff = (bq.astype(f32) + b @ wq).reshape(NCH, 128).T.copy()          # [128, NCH]
    bk_eff = ((bk.astype(f32) + b @ wk) * 0.125).reshape(NCH, 128).T.copy()
    # relpad[j, h*32+q] = rel_bias[h, q, j] for j<32, zero-padded to 128 rows
    relpad = np.zeros((128, H * 32), dtype=f32)
    relpad[:32] = rel_bias.astype(f32).transpose(2, 0, 1).reshape(32, H * 32)
    exprel = np.tile(
        np.exp(rel_bias.astype(f32)).transpose(2, 0, 1).reshape(32, H * 32), (4, 1)
    )
    ident = np.eye(128, dtype=f32).astype(BF16)
    ident8 = np.eye(128, dtype=f32).astype(F8)
    id4pad = np.zeros((128, 128), dtype=f32)
    id4pad[:32] = np.tile(np.eye(32, dtype=f32), (1, 4))
    bdones = np.kron(np.eye(4, dtype=f32), np.ones((32, 32), dtype=f32))
    # bv/bo/ln_b contributions that survive softmax-normalization exactly:
    # out += ((ln_b@wv + bv) @ wo + bo). Zero for this problem's fills.
    c0 = (b @ wv + bv.astype(f32)) @ wo + bo.astype(f32)
    return dict(
        wqs=wqs, wks=wks, wvs=wvs, wos=wos,
        bq_eff=np.ascontiguousarray(bq_eff), bk_eff=np.ascontiguousarray(bk_eff),
        relpad=relpad.astype(BF16), ident=ident, ident8=ident8,
        exprel=exprel.astype(BF16),
        id4pad=id4pad.astype(BF16), bdones=bdones.astype(BF16),
    ), c0


_BUILT = {}


def _get_nc(repeat: int = 1):
    if repeat not in _BUILT:
        _BUILT[repeat] = build_nc(repeat)
    return _BUILT[repeat]


def make_in_maps(inputs: dict, consts: dict) -> list:
    x = np.asarray(inputs["x"], dtype=np.float32).reshape(B * S, D).astype(BF16)
    in_maps = []
    for c in range(N_CORES):
        m = dict(consts)
        m["x"] = np.ascontiguousarray(x[c * TPC : (c + 1) * TPC])
        in_maps.append(m)
    return in_maps


def kernel(**inputs) -> np.ndarray:
    from concourse.bass_utils import run_bass_kernel_spmd

    consts, c0 = _host_constants(
        inputs["ln_g"], inputs["ln_b"], inputs["wq"], inputs["bq"],
        inputs["wk"], inputs["bk"], inputs["wv"], inputs["bv"],
        inputs["wo"], inputs["bo"], inputs["rel_bias"],
    )
    nc = _get_nc(1)
    in_maps = make_in_maps(inputs, consts)
    res = run_bass_kernel_spmd(nc, in_maps, core_ids=list(range(N_CORES)), trace=False)
    out = np.concatenate(
        [np.asarray(res.results[c]["y"], dtype=np.float32) for c in range(N_CORES)],
        axis=0,
    )
    out = out.reshape(B, S, D)
    if np.any(c0 != 0.0):
        out = out + c0.astype(np.float32)
    return out



# revision 57
# speedup vs baseline: 1.0859x; 1.0859x over previous
"""Trainium2 Bass kernel for nn_MultiHeadSelfAttentionModule_6193342840934.

Reference math (per batch row b of x[B,S,D]):
    xn  = LayerNorm(x) * ln_g + ln_b
    Q/K/V = xn @ w{q,k,v} + b{q,k,v}   (heads H=16, dk=64)
    scores = Q K^T / sqrt(dk) + rel_bias[h]          (S=32)
    out = x + softmax(scores) @ V @ wo + bo

Distribution: pure data-parallel over the batch dim, 2048/8 = 256 batches
(8192 tokens) per NeuronCore. Weights are replicated to every core.

Per-core kernel layout strategy:
  - tokens processed in 512-token super-tiles (16 per core); x uploaded
    bf16 and y returned bf16 (host casts) — halves both HBM streams, the
    bf16 noise is far below the fp8 noise already in the matmul path.
  - all four D x D projections run as fp8e4 DoubleRow matmuls (both
    operands fp8, 256-deep contraction per matmul, 2x PE throughput;
    weights quantized host-side, xn/ctx quantized at evacuation).
    End-to-end rel err ~7.8e-3 on HW.
  - LayerNorm token-major via bn_stats; ln_g/ln_b folded into the weights
    host-side (exact): wq' = ln_g*wq, bq' = bq + ln_b@wq, etc. rsig
    computed as exp(-0.5*ln(var+eps)) so the ONLY ACT functions used are
    {identity, copy, exp, ln} — one activation table, zero table-swap
    stalls. The LN apply itself runs on DVE (single-src tensor_scalar
    with per-partition AP scale/bias), keeping the ACT engine off the
    projection-phase critical path.
  - xn transposed to d-major via regular fp8 matmuls against an fp8
    identity (PE transpose-mode is slower per 128x128 on HW).
  - scores computed TRANSPOSED (scoresT[k,q] = K'^T Q) so that softmax's
    denominator can be computed with a block-diagonal-ones matmul over the
    partition dim and the attention application needs no transpose of the
    attention matrix. 1/sqrt(dk) folded into K evacuation, rel_bias added
    via an identity-matmul accumulated into the same PSUM region.
  - IMPORTANT HW CONSTRAINT (found empirically on this runtime): consecutive
    matmuls whose operands sit at different base partitions (different PE
    row groups) crash the device. Every matmul therefore uses the full
    128-partition contraction; sub-128 contractions are expressed by
    zero-padding one operand (exact in fp: 0*x = 0), and only the PSUM
    column position varies via tile_position.
  - Q lands in a per-head-pair BLOCK-DIAGONAL layout qbd (built by
    SBUF->SBUF DMA from the dense evac; DMA engines otherwise idle):
    heads 2c/2c+1 occupy partition halves of slot c with zeros elsewhere,
    so ONE score matmul per (head-pair, batch) with the dense K tile as
    lhsT computes both heads over the full 128-partition contraction —
    half the score-MM count and half the LDWEIGHTS columns of a per-head
    zero-padded layout. qbd/at_bd zero regions are persistent, memset once.
  - softmax without max-subtraction (scores are O(10) here; exp is safe in
    fp32); denominators via ones-matmul; 1/denom via exp(-ln(x)) on ACT,
    emitted in bf16 so the at_bd normalization multiplies run in DVE
    2x_1P mode; the normalization writes the per-batch block-diagonal
    at_bd tiles directly (no separate attn tile or copies).
  - phase-pipelined schedule: scores+exp for all 4 sub-tiles stream first;
    the next super-tile's LN/transposes + V projection (V only needs the
    current sub-tile's xnT columns as the stationary operand, so it runs
    per sub-tile right after that sub-tile's transposes) occupy the PE
    while this super-tile's softmax-normalization rides between those
    matmul groups; Q/K projections follow with the full xnT; pass_out
    (AV + output projection + residual) runs last with everything ready.
  - ctx computed d-major, 16 AV matmuls split across two single-bank PSUM
    tiles in a 2-deep ring (next sub-tile's AV never waits on this one's
    evacuation), output projection token-major, residual added by the
    PSUM-evacuating tensor_add.
"""

import os

import numpy as np
import ml_dtypes

# timing-attribution ablations (wrong numerics, same dataflow shape):
#   BASS_MHSA_ABLATE=scores4   only every 4th head's score matmuls
#   BASS_MHSA_ABLATE=noctx     1 of the 16 per-head AV matmuls
#   BASS_MHSA_ABLATE=bf16proj  projections as bf16 pairs, no fp8 DoubleRow
#   BASS_MHSA_ABLATE=tmode     PE transpose-mode instead of normal fp8 matmuls
ABLATE = os.environ.get("BASS_MHSA_ABLATE", "")

import concourse.bass as bass
import concourse.tile as tile
import concourse.mybir as mybir
from concourse.alu_op_type import AluOpType
from concourse.vector_clock import ScopedClock

dt = mybir.dt
AF = mybir.ActivationFunctionType
PM = mybir.MatmulPerfMode

B, S, D, H = 2048, 32, 1024, 16
DK = D // H          # 64
EPS = 1e-5
N_CORES = 8
BPC = B // N_CORES   # 256 batches per core
TPC = BPC * S        # 8192 tokens per core
ST = 512             # tokens per super-tile
NSUB = ST // 128     # 4 sub-tiles of 128 tokens
NSUP = TPC // ST     # 16 super-tiles
NCH = D // 128       # 8 d-chunks

BF16 = ml_dtypes.bfloat16
F8 = ml_dtypes.float8_e4m3


class SplitDrainTileContext(tile.TileContext):
    """This container's walrus build rejects >1 sync-wait on a Drain
    instruction; split the tail drain's waits across standalone NOPs."""

    def _drain_and_barrier(self, tick_clock, wait_clock):
        drain_inst = self.nc.sync.drain()
        wait_clock.add_sem_waits(
            drain_inst.ins, ScopedClock({None: tick_clock.global_clock})
        )
        si = drain_inst.ins.sync_info
        waits = list(si.on_wait or []) if si is not None else []
        if len(waits) > 1:
            drain_inst.ins.sync_info.on_wait = waits[:1]
            for w in waits[1:]:
                nop = self.nc.sync.nop(hint="drain_split_wait", nofuse=True)
                nop.ins.sync_info = mybir.SyncInfo(on_wait=[w], on_update=[])
        self.nc.all_engine_barrier()
        assert self.sems is not None
        popped = self.nc._tile_sem_poison_stack.pop()
        assert popped is self._sem_poison
        self.nc.clear_and_free_semaphores(list(self.sems.allocated().values()))
        self.nc.all_engine_barrier()


def _split_excess_waits(nc: bass.Bass):
    """This container's walrus accepts at most 1 sync-wait per instruction
    (2 for EventSemaphore), but this tile version assigns up to 4. Move
    excess waits onto injected same-engine NoOps right before the
    instruction — engine streams are in-order, so this is equivalent."""
    for f in nc.m.functions:
        for bb in f.blocks:
            insts = list(bb.instructions)
            out = []
            changed = False
            for inst in insts:
                si = inst.sync_info
                cap = 2 if inst.opcode == "EventSemaphore" else 1
                waits = list(si.on_wait) if si is not None and si.on_wait else []
                if len(waits) > cap:
                    changed = True
                    for w in waits[cap:]:
                        nop = mybir.InstNoOp(
                            name=nc.get_next_instruction_name(),
                            engine=inst.engine,
                            sync_info=mybir.SyncInfo(on_wait=[w], on_update=[]),
                            bass_nofuse=True,
                        )
                        out.append(nop)
                    inst.sync_info = mybir.SyncInfo(
                        on_wait=waits[:cap], on_update=list(si.on_update or [])
                    )
                out.append(inst)
            if changed:
                bb.instructions = out


def build_nc(repeat: int = 1, split_waits: bool = True) -> bass.Bass:
    """Build the per-core Bass module. repeat>1 wraps the body in a hardware
    loop (used only for benchmarking slope timing). split_waits applies the
    walrus 1-wait-per-instruction workaround (disable for CoreSim runs)."""
    nc = bass.Bass("TRN2", target_bir_lowering=False, debug=False, num_devices=1)

    f32 = dt.float32
    bf16 = dt.bfloat16
    f8 = dt.float8e4
    wddt = bf16 if ABLATE == "bf16proj" else f8

    # x is uploaded as bf16 (host-side cast): halves the input HBM traffic;
    # bn_stats/LN-apply/residual-add all tolerate the 0.4% bf16 noise (the
    # attention path quantizes to fp8 anyway and the residual error is well
    # inside the rel-err budget).
    x_d = nc.dram_tensor("x", [TPC, D], bf16, kind="ExternalInput").ap()
    # y in bf16: halves the output HBM traffic; the host upcasts to f32.
    y_d = nc.dram_tensor("y", [TPC, D], bf16, kind="ExternalOutput").ap()
    # weights are pre-laid-out host-side so every matmul's weight slice is
    # CONTIGUOUS per partition (the PE's weight-load path streams strided
    # APs slower): wq/wk as [p, c_out, cp, two, 128] (lhsT blocks), wv/wo
    # as [p, cp, half, two, 512] (rhs blocks).
    wq_d = nc.dram_tensor(
        "wqs", [128, NCH, NCH // 2, 2, 128], wddt, kind="ExternalInput").ap()
    wk_d = nc.dram_tensor(
        "wks", [128, NCH, NCH // 2, 2, 128], wddt, kind="ExternalInput").ap()
    wv_d = nc.dram_tensor(
        "wvs", [128, NCH // 2, 2, 2, 512], wddt, kind="ExternalInput").ap()
    wo_d = nc.dram_tensor(
        "wos", [128, NCH // 2, 2, 2, 512], wddt, kind="ExternalInput").ap()
    bq_d = nc.dram_tensor("bq_eff", [128, NCH], f32, kind="ExternalInput").ap()
    bk_d = nc.dram_tensor("bk_eff", [128, NCH], f32, kind="ExternalInput").ap()
    # relpad[j, h*32+q] = rel_bias[h, q, j] for j<32, 0 for j>=32
    rel_d = nc.dram_tensor("relpad", [128, H * 32], bf16, kind="ExternalInput").ap()
    id_d = nc.dram_tensor("ident", [128, 128], bf16, kind="ExternalInput").ap()
    id8_d = nc.dram_tensor("ident8", [128, 128], f8, kind="ExternalInput").ap()
    # id4pad[j, p] = (j == p % 32) for j<32, 0 for j>=32
    id4_d = nc.dram_tensor("id4pad", [128, 128], bf16, kind="ExternalInput").ap()
    # exprel[(b,k), (h,q)] = exp(rel_bias[h, q, k])  (tiled over the 4 blocks)
    exprel_d = nc.dram_tensor("exprel", [128, H * 32], bf16, kind="ExternalInput").ap()
    # bdones[(b,k), (b',m)] = (b == b')  (32-block diagonal of ones)
    bdon_d = nc.dram_tensor("bdones", [128, 128], bf16, kind="ExternalInput").ap()

    with SplitDrainTileContext(nc) as tc:
        with (
            tc.tile_pool(name="consts", bufs=1) as consts,
            tc.tile_pool(name="xin", bufs=(6 if ABLATE == "bf16proj" else 16)) as xin_pool,
            tc.tile_pool(name="small", bufs=8) as small,
            tc.tile_pool(name="xn0", bufs=2) as xn0_pool,
            tc.tile_pool(name="xnT", bufs=2) as xnT_pool,
            tc.tile_pool(name="qk", bufs=2) as qk_pool,
            tc.tile_pool(name="vsb", bufs=2) as v_pool,
            tc.tile_pool(name="attn", bufs=2) as attn_pool,
            tc.tile_pool(name="atu", bufs=4) as atu_pool,
            tc.tile_pool(name="ctx", bufs=2) as ctx_pool,
            tc.tile_pool(name="osb", bufs=(1 if ABLATE == "bf16proj" else 2)) as out_pool,
            tc.tile_pool(name="ps_proj", bufs=2, space="PSUM") as ps_proj,
            tc.tile_pool(name="ps_attn", bufs=2, space="PSUM") as ps_attn,
            tc.tile_pool(name="ps_ctx", bufs=2, space="PSUM") as ps_ctx,
            tc.tile_pool(name="ps_xp", bufs=2, space="PSUM") as ps_xp,
        ):
            # -- resident constants -------------------------------------------
            wdt = bf16 if ABLATE == "bf16proj" else f8
            wq_s = consts.tile([128, NCH, NCH // 2, 2, 128], wdt)
            wk_s = consts.tile([128, NCH, NCH // 2, 2, 128], wdt)
            wv_s = consts.tile([128, NCH // 2, 2, 2, 512], wdt)
            wo_s = consts.tile([128, NCH // 2, 2, 2, 512], wdt)
            for wsb, wd in ((wq_s, wq_d), (wk_s, wk_d), (wv_s, wv_d), (wo_s, wo_d)):
                nc.sync.dma_start(wsb, wd)
            bq_s = consts.tile([128, NCH], f32)
            nc.sync.dma_start(bq_s, bq_d)
            bk_s = consts.tile([128, NCH], f32)
            nc.sync.dma_start(bk_s, bk_d)
            rel_s = consts.tile([128, H * 32], bf16)
            nc.sync.dma_start(rel_s, rel_d)
            id_s = consts.tile([128, 128], bf16)
            nc.sync.dma_start(id_s, id_d)
            id8_s = consts.tile([128, 128], f8)
            nc.sync.dma_start(id8_s, id8_d)
            id4_s = consts.tile([128, 128], bf16)
            nc.sync.dma_start(id4_s, id4_d)
            exprel_s = consts.tile([128, H * 32], bf16)
            nc.sync.dma_start(exprel_s, exprel_d)
            bdon_s = consts.tile([128, 128], bf16)
            nc.sync.dma_start(bdon_s, bdon_d)
            eps_s = consts.tile([128, 1], f32)
            nc.vector.memset(eps_s, EPS)

            # persistent zero-padded tiles (double-buffered by hand): the
            # zero regions are written once here and never touched again —
            # evacuations only write the valid blocks, so the per-super-tile
            # gpsimd memsets of the baseline are hoisted out of the loop.
            # qbd holds Q in a per-head-pair BLOCK-DIAGONAL layout:
            # qbd[0:64, c, 0, :] = Q of head 2c, qbd[64:128, c, 1, :] = Q of
            # head 2c+1, zeros elsewhere — so ONE score matmul per (pair,
            # batch) with dense ks as lhsT computes both heads' scores over
            # the full 128-partition contraction (zeros kill the cross-head
            # terms). Halves both the score-MM count and the LDWEIGHTS
            # column traffic vs the per-head zero-padded K layout.
            qbds = []
            for i in range(1 if ABLATE == "bf16proj" else 2):
                qb = consts.tile([128, NCH, 2, ST], bf16, tag=f"qbdp{i}")
                nc.gpsimd.memset(qb, 0.0)
                qbds.append(qb)
            # one at_bd per sub-tile: the norm pass (interleaved into the
            # next supertile's projections) writes all four before pass_out
            # reads any — a 2-deep ring would overwrite live data.
            at_bds = []
            for i in range(NSUB):
                ab = consts.tile([128, H, 128], bf16, tag=f"atbdp{i}")
                nc.gpsimd.memset(ab, 0.0)
                at_bds.append(ab)

            # per-super-tile prelude state (xts list + mv4 + xnT tile).
            # prelude_load(sup, s) DMAs one 128-token tile and computes its
            # bn stats into a shared [128, 4, 2] tile; prelude_finish(sup)
            # batches the LN scalar chain across all 4 tiles (4x fewer tiny
            # ops), applies LN to fp8, and transposes via regular fp8
            # matmuls against an fp8 identity (the PE transpose-mode path
            # is ~2-3x slower per 128x128 on HW than a plain N=128 matmul).
            state: dict = {}

            def prelude_load(sup: int, s: int):
                t0 = sup * ST
                if s == 0:
                    xnT = xnT_pool.tile(
                        [128, NCH, ST], bf16 if ABLATE == "bf16proj" else f8, tag="xnT"
                    )
                    mv4 = small.tile([128, 4, 2], f32, tag="mv4")
                    state[sup] = ([], xnT, mv4)
                xts, xnT, mv4 = state[sup]
                row = t0 + s * 128
                xt = xin_pool.tile([128, D], bf16, tag="x")
                nc.sync.dma_start(xt, x_d[row : row + 128, :])
                xts.append(xt)
                st6 = small.tile([128, 2, 6], f32, tag="st6")
                nc.vector.bn_stats(st6[:, 0, :], xt[:, 0:512])
                nc.vector.bn_stats(st6[:, 1, :], xt[:, 512:1024])
                nc.vector.bn_aggr(mv4[:, s, :], st6)

            lnstate: dict = {}

            def prelude_ln(sup: int, s: int):
                """LN scalar chain (s==0) + sub-tile s's LN apply for
                supertile sup, hoisted into the PREVIOUS fused pass: the ACT
                ln/exp + ALL FOUR DVE LN-applies complete while the PE
                streams that pass's matmuls, so prelude_finish(sup)'s
                transposes are pure PE work with zero DVE dependency.
                rsig = exp(-0.5*ln(var+eps)): keeps every ACT function used
                by this kernel (identity/copy/exp/ln) inside ONE activation
                table — an AF.Sqrt here would force two table swaps per
                super-tile right in the softmax chain's way."""
                xts, _, mv4 = state[sup]
                if s == 0:
                    lnv4 = small.tile([128, 4], f32, tag="lnv4")
                    nc.scalar.activation(lnv4, mv4[:, :, 1], AF.Ln, bias=eps_s[:])
                    rsig4 = small.tile([128, 4], f32, tag="rsig4")
                    nc.scalar.activation(rsig4, lnv4, AF.Exp, scale=-0.5)
                    nmu4 = small.tile([128, 4], f32, tag="nmu4")
                    nc.vector.tensor_mul(nmu4, mv4[:, :, 0], rsig4)
                    nmr4 = small.tile([128, 4], f32, tag="nmr4")
                    nc.vector.tensor_scalar_mul(nmr4, nmu4, -1.0)
                    lnstate[sup] = (rsig4, nmr4, [])
                rsig4, nmr4, xn0s = lnstate[sup]
                xn0 = xn0_pool.tile([128, D], f8, tag="xn0")
                nc.vector.tensor_scalar(
                    xn0, xts[s], rsig4[:, s : s + 1], nmr4[:, s : s + 1],
                    AluOpType.mult, AluOpType.add,
                )
                xn0s.append(xn0)

            def prelude_finish(sup: int):
                """LN scalar chain + per-subtile: LN-apply, transposes, V
                projection, and (interleaved) norm_sup's softmax
                normalization. V only needs THIS subtile's xnT columns (it is
                the matmul's stationary side), so it runs right after the
                subtile's transposes — the PE gets dense matmul work per
                subtile instead of idling until the whole xnT tile is built,
                and the norm chain rides between those matmul groups."""
                xts, xnT, mv4 = state[sup]
                _, _, xn0s = lnstate.pop(sup)
                vs = v_pool.tile([128, NSUB, D], bf16, tag="v")
                vstate[sup] = vs

                def ln_transpose(s: int):
                    # all four LN applies were hoisted into the previous
                    # fused pass (prelude_ln) — the transposes are pure PE
                    # work with zero DVE dependency at prelude start.
                    xn0 = xn0s[s]
                    for half in range(2):
                        if ABLATE == "tmode":
                            xp = ps_xp.tile([128, NCH // 2, 128], f8, tag="xp")
                            for c in range(NCH // 2):
                                cc = half * (NCH // 2) + c
                                nc.tensor.transpose(
                                    xp[:, c, :], xn0[:, cc * 128 : (cc + 1) * 128], id8_s
                                )
                        else:
                            xp = ps_xp.tile([128, NCH // 2, 128], f32, tag="xp")
                            for c in range(NCH // 2):
                                cc = half * (NCH // 2) + c
                                nc.tensor.matmul(
                                    xp[:, c, :],
                                    lhsT=xn0[:, cc * 128 : (cc + 1) * 128],
                                    rhs=id8_s,
                                    start=True,
                                    stop=True,
                                    skip_group_check=True,
                                )
                        # alternate the PSUM->SBUF evacuation between DVE and
                        # ACT: DVE is the busiest engine, ACT has slack.
                        evac = nc.vector.tensor_copy if half == 0 else (
                            lambda o, i: nc.scalar.activation(o, i, AF.Copy)
                        )
                        evac(
                            xnT[:, half * (NCH // 2) : (half + 1) * (NCH // 2),
                                s * 128 : (s + 1) * 128],
                            xp,
                        )

                # transposes run one sub-tile AHEAD of the V projections so
                # the V matmuls never wait on their own sub-tile's xp
                # evacuations — the PE is chewing s+1's transposes while
                # s's evacs drain.
                ln_transpose(0)
                for s in range(NSUB):
                    if s + 1 < NSUB:
                        ln_transpose(s + 1)
                    # V projection for sub-tile s (uses only sub-tile s's
                    # xnT columns as the stationary operand).
                    for half in range(2):
                        ps = ps_proj.tile([128, 512], f32, tag="proj")
                        contract_x(ps, xnT, slice(s * 128, (s + 1) * 128), wv_s, half)
                        # alternate V evacs between ACT and DVE: DVE is the
                        # prelude-phase bottleneck, and the V evac gates the
                        # Q projections via the PSUM ring.
                        if half == 0:
                            nc.scalar.activation(
                                vs[:, s, 0:512], ps, AF.Copy
                            )
                        else:
                            nc.vector.tensor_copy(vs[:, s, 512:1024], ps)

            def contract_w(ps, w_r, c, xnT):
                """Q/K d-contraction: weight-stationary, lhsT = contiguous
                [p, 2, 128] blocks of the re-laid weight, rhs = xnT chunk
                pairs (contiguous). fp8 DoubleRow, or bf16 under ablation."""
                if ABLATE == "bf16proj":
                    for ci in range(NCH):
                        nc.tensor.matmul(
                            ps,
                            lhsT=w_r[:, c, ci // 2, ci % 2, :],
                            rhs=xnT[:, ci, :],
                            start=(ci == 0),
                            stop=(ci == NCH - 1),
                        )
                else:
                    ncp = NCH // 4 if ABLATE == "projhalf" else NCH // 2
                    for cp in range(ncp):
                        nc.tensor.matmul(
                            ps,
                            lhsT=w_r[:, c, cp, :, :],
                            rhs=xnT[:, 2 * cp : 2 * cp + 2, :],
                            start=(cp == 0),
                            stop=(cp == ncp - 1),
                            perf_mode=PM.DoubleRow,
                        )

            def contract_x(ps, lhs_tile, lhs_cols, w_r, half):
                """V/O d-contraction: activation-stationary, rhs = contiguous
                [p, 2, 512] blocks of the re-laid weight."""
                if ABLATE == "bf16proj":
                    for ci in range(NCH):
                        nc.tensor.matmul(
                            ps,
                            lhsT=lhs_tile[:, ci, lhs_cols],
                            rhs=w_r[:, ci // 2, half, ci % 2, :],
                            start=(ci == 0),
                            stop=(ci == NCH - 1),
                        )
                else:
                    ncp = NCH // 4 if ABLATE == "projhalf" else NCH // 2
                    for cp in range(ncp):
                        nc.tensor.matmul(
                            ps,
                            lhsT=lhs_tile[:, 2 * cp : 2 * cp + 2, lhs_cols],
                            rhs=w_r[:, cp, half, :, :],
                            start=(cp == 0),
                            stop=(cp == ncp - 1),
                            perf_mode=PM.DoubleRow,
                        )

            # per-supertile attention state carried between the pass functions
            astate: dict = {}
            vstate: dict = {}

            def emit_norm(sup: int, s: int):
                """Softmax denominator + reciprocal + block-diagonalization
                for one sub-tile. Emitted interleaved between projection
                chunk groups so the PE always has matmul work while the
                ACT ln/exp + DVE multiply chain completes."""
                at_us = astate[sup][3]
                # per-batch-block softmax denominators, replicated across
                # each 32-row block by the block-diagonal ones matmul
                dn = ps_attn.tile([128, H * 32], f32, tag="attn")
                nc.tensor.matmul(dn, lhsT=bdon_s, rhs=at_us[s], start=True, stop=True)
                # 1/denom via exp(-ln(x)) on ACT: this walrus build rejects
                # the custom-DVE fast-reciprocal ISA op, and the native DVE
                # reciprocal is ~8 cyc/elem. LUT rel-err ~1e-4 is fine at
                # bf16 noise levels.
                lnd = attn_pool.tile([128, H * 32], f32, tag="lnd")
                nc.scalar.activation(lnd, dn, AF.Ln)
                # rc in bf16: the at_bd normalization multiplies below then
                # run in DVE 2x_1P mode (both operands 16-bit) — ~2x faster.
                rc = attn_pool.tile([128, H * 32], bf16, tag="rc")
                nc.scalar.activation(rc, lnd, AF.Exp, scale=-1.0)
                # normalize (at_u * rc) fused directly into the per-batch
                # block-diagonal writes: at_bd[(b,k), h, (b,q)] nonzero
                # only for matching b, so AV can contract over the full
                # 128 token partitions. Zero regions are persistent.
                at_bd = at_bds[s % len(at_bds)]
                atuv = at_us[s].rearrange("p (h q) -> p h q", h=H)
                rcv = rc.rearrange("p (h q) -> p h q", h=H)
                for b in range(4):
                    blk = slice(b * 32, (b + 1) * 32)
                    nc.vector.tensor_mul(
                        at_bd[blk, :, blk], atuv[blk, :, :], rcv[blk, :, :]
                    )

            def proj(sup: int):
                """Q/K projections for supertile sup (fp8 DoubleRow); V runs
                inside prelude_finish."""
                _, xnT, _ = state[sup]
                qs = qk_pool.tile([128, NCH, ST], bf16, tag="q")
                ks = qk_pool.tile([128, NCH, ST], bf16, tag="k")
                qbd = qbds[sup % len(qbds)]
                astate[sup] = (qbd, ks, vstate.pop(sup))
                for c in range(NCH):
                    ps = ps_proj.tile([128, 512], f32, tag="proj")
                    contract_w(ps, wq_s, c, xnT)
                    # full-partition evac to dense qs; the block-diagonal qbd
                    # layout is built by SBUF->SBUF DMA (DMA engines are
                    # otherwise mostly idle), staged per 4 chunks to hide the
                    # copy latency behind the remaining projections.
                    nc.scalar.activation(
                        qs[:, c, :], ps, AF.Identity, bias=bq_s[:, c : c + 1], scale=1.0
                    )
                    if c == 3 or c == 7:
                        c0 = c - 3
                        nc.sync.dma_start(
                            qbd[0:64, c0 : c + 1, 0, :], qs[0:64, c0 : c + 1, :]
                        )
                        nc.sync.dma_start(
                            qbd[64:128, c0 : c + 1, 1, :], qs[64:128, c0 : c + 1, :]
                        )
                for c in range(NCH):
                    ps = ps_proj.tile([128, 512], f32, tag="proj")
                    contract_w(ps, wk_s, c, xnT)
                    nc.scalar.activation(
                        ks[:, c, :], ps, AF.Identity,
                        bias=bk_s[:, c : c + 1], scale=0.125,
                    )

            def scores_subtile(sup: int, s: int):
                """Scores + exp for ONE sub-tile of supertile sup.
                scoresT[(b,k), (h,q)] = K'^T Q + rel_biasT  (PSUM bank).
                NB: skip_group_check — the sim's coarse PSUM zero-region
                bookkeeping can't express "one full-region start, many
                sub-block accumulates"; on HW this is per-element
                has_written and PE executes in program order."""
                st = astate[sup]
                if len(st) == 3:
                    astate[sup] = st + ([],)
                qbd, ks, _, at_us = astate[sup]
                sc = ps_attn.tile([128, H * 32], f32, tag="attn")
                nc.tensor.matmul(
                    sc, lhsT=id4_s, rhs=rel_s, start=True, stop=False,
                    skip_group_check=True,
                )
                # one matmul per (head-pair, batch): lhsT = dense ks
                # (heads 2c/2c+1 stacked on the partition dim), rhs = the
                # block-diagonal qbd slice [128, 2, 32] -> 64 output cols
                # (j2, q) landing exactly on the (h, q) column layout.
                sc_pairs = (
                    list(range(0, NCH, 4)) if ABLATE == "scores4" else list(range(NCH))
                )
                for ci, c in enumerate(sc_pairs):
                    for b in range(4):
                        tok = slice(s * 128 + b * 32, s * 128 + (b + 1) * 32)
                        nc.tensor.matmul(
                            sc[b * 32 : (b + 1) * 32, c * 64 : (c + 1) * 64],
                            lhsT=ks[:, c, tok],
                            rhs=qbd[:, c, :, tok],
                            start=False,
                            stop=(ci == len(sc_pairs) - 1),
                            tile_position=(0, b * 32),
                            skip_group_check=True,
                        )
                at_u = atu_pool.tile([128, H * 32], bf16, tag="atu")
                nc.scalar.activation(at_u, sc, AF.Exp)
                at_us.append(at_u)

            def pass_out(sup: int, nxt: int | None = None):
                """AV matmuls + output projection + residual, per sub-tile,
                with the NEXT supertile's score matmuls, softmax
                normalization, and prefetch loads interleaved: this pass is
                chain-paced with PE slack, while prelude+projection is
                PE-paced — so ALL movable work (scores, the dn matmul, the
                ACT ln/exp + DVE at_bd chain) rides here. The at_bd[s] write
                of norm(nxt) lands right after this pass's own AV(s) read of
                the same shared tile."""
                t0 = sup * ST
                xts = state[sup][0]
                vs = astate[sup][2]
                for s in range(NSUB):
                    # ctxT[(h,dv), t] d-major: one matmul per head over all 4
                    # batches at once (cross-batch terms killed by at_bd
                    # zeros). Heads are split across TWO single-bank PSUM
                    # tiles in a 2-deep ring so the next sub-tile's AV
                    # matmuls never stall on this sub-tile's evacuation.
                    at_bd = at_bds[s % len(at_bds)]
                    ctxT = ctx_pool.tile(
                        [128, NCH, 128], bf16 if ABLATE == "bf16proj" else f8, tag="ctxT"
                    )
                    nheads = 1 if ABLATE == "noctx" else H
                    for g in range(2):
                        cps = ps_ctx.tile([128, 4, 128], f32, tag="ctx")
                        for h in range(g * 8, min((g + 1) * 8, nheads)):
                            pb = (h % 2) * 64
                            nc.tensor.matmul(
                                cps[pb : pb + 64, (h // 2) % 4, :],
                                lhsT=vs[:, s, h * 64 : (h + 1) * 64],
                                rhs=at_bd[:, h, :],
                                start=True,
                                stop=True,
                                tile_position=(0, pb),
                                skip_group_check=True,
                            )
                        if nheads > g * 8 or g == 0:
                            nc.scalar.activation(
                                ctxT[:, g * 4 : (g + 1) * 4, :], cps, AF.Copy
                            )

                    # next supertile's score matmuls ride here: the PE has
                    # ~2.7us of independent work while ctxT evacuates.
                    if nxt is not None:
                        scores_subtile(nxt, s)
                        # hoist supertile nxt+1's LN chain + LN applies
                        # under this pass's matmul stream (see prelude_ln).
                        if nxt + 1 < NSUP:
                            prelude_ln(nxt + 1, s)

                    # out = x + ctx @ wo   (token-major, fp8 DoubleRow)
                    outsb = out_pool.tile([128, D], bf16, tag="osb")
                    for half in range(2):
                        ps = ps_proj.tile([128, 512], f32, tag="proj")
                        contract_x(ps, ctxT, slice(None), wo_s, half)
                        nc.vector.tensor_add(
                            outsb[:, half * 512 : (half + 1) * 512],
                            xts[s][:, half * 512 : (half + 1) * 512],
                            ps,
                        )
                    row = t0 + s * 128
                    nc.sync.dma_start(y_d[row : row + 128, :], outsb)
                    if nxt is not None:
                        emit_norm(nxt, s)
                    # x loads + bn stats for supertile nxt+2 are emitted
                    # AFTER this sub-tile's residual add (the load's xin buf
                    # is freed by an earlier add, keeping the DVE queue
                    # deadlock-free) — still a full pipeline iteration ahead
                    # of prelude_finish(nxt+2)'s LN chain.
                    if nxt is not None and nxt + 2 < NSUP:
                        prelude_load(nxt + 2, s)
                del astate[sup]
                del state[sup]

            def run_all():
                # pipeline: while the PE chews supertile sup+1's projections,
                # the ACT/DVE softmax chain of supertile sup completes, so
                # pass_norm/pass_out never stall the PE stream for long.
                for s in range(NSUB):
                    prelude_load(0, s)
                for s in range(NSUB):
                    prelude_ln(0, s)
                prelude_finish(0)
                for s in range(NSUB):
                    prelude_load(1, s)
                for s in range(NSUB):
                    prelude_ln(1, s)
                proj(0)
                # first supertile's scores run standalone (no pass_out to
                # fuse into); supertile 2's loads ride along.
                for s in range(NSUB):
                    prelude_load(2, s)
                    scores_subtile(0, s)
                    emit_norm(0, s)
                for sup in range(NSUP):
                    if sup + 1 < NSUP:
                        prelude_finish(sup + 1)
                        proj(sup + 1)
                        pass_out(sup, nxt=sup + 1)
                    else:
                        pass_out(sup, nxt=None)

            if repeat > 1:
                with tc.For_i(0, repeat, 1):
                    run_all()
            else:
                run_all()

    if split_waits:
        _split_excess_waits(nc)
    return nc


def _host_constants(ln_g, ln_b, wq, bq, wk, bk, wv, bv, wo, bo, rel_bias):
    """Exact host-side weight transforms (fold LN affine + 1/sqrt(dk))."""
    f32 = np.float32
    g = ln_g.astype(f32)
    b = ln_b.astype(f32)
    wq = wq.astype(f32)
    wk = wk.astype(f32)
    wv = wv.astype(f32)
    wo = wo.astype(f32)
    wdt = BF16 if ABLATE == "bf16proj" else F8
    # re-layout so every on-device weight slice is contiguous per partition:
    #   wq/wk: [p, c_out, cp, two, 128]  (lhsT blocks, d_in=(cp,two,p))
    #   wv/wo: [p, cp, half, two, 512]   (rhs blocks)
    wqs = np.ascontiguousarray(
        (g[:, None] * wq).astype(wdt).reshape(4, 2, 128, 8, 128)
        .transpose(2, 3, 0, 1, 4)
    )
    wks = np.ascontiguousarray(
        (g[:, None] * wk).astype(wdt).reshape(4, 2, 128, 8, 128)
        .transpose(2, 3, 0, 1, 4)
    )
    wvs = np.ascontiguousarray(
        (g[:, None] * wv).astype(wdt).reshape(4, 2, 128, 2, 512)
        .transpose(2, 0, 3, 1, 4)
    )
    wos = np.ascontiguousarray(
        wo.astype(wdt).reshape(4, 2, 128, 2, 512).transpose(2, 0, 3, 1, 4)
    )
    bq_eff = (bq.astype(f32) + b @ wq).reshape(NCH, 128).T.copy()          # [128, NCH]
    bk_eff = ((bk.astype(f32) + b @ wk) * 0.125).reshape(NCH, 128).T.copy()
    # relpad[j, h*32+q] = rel_bias[h, q, j] for j<32, zero-padded to 128 rows
    relpad = np.zeros((128, H * 32), dtype=f32)
    relpad[:32] = rel_bias.astype(f32).transpose(2, 0, 1).reshape(32, H * 32)
    exprel = np.tile(
        np.exp(rel_bias.astype(f32)).transpose(2, 0, 1).reshape(32, H * 32), (4, 1)
    )
    ident = np.eye(128, dtype=f32).astype(BF16)
    ident8 = np.eye(128, dtype=f32).astype(F8)
    id4pad = np.zeros((128, 128), dtype=f32)
    id4pad[:32] = np.tile(np.eye(32, dtype=f32), (1, 4))
    bdones = np.kron(np.eye(4, dtype=f32), np.ones((32, 32), dtype=f32))
    # bv/bo/ln_b contributions that survive softmax-normalization exactly:
    # out += ((ln_b@wv + bv) @ wo + bo). Zero for this problem's fills.
    c0 = (b @ wv + bv.astype(f32)) @ wo + bo.astype(f32)
    return dict(
        wqs=wqs, wks=wks, wvs=wvs, wos=wos,
        bq_eff=np.ascontiguousarray(bq_eff), bk_eff=np.ascontiguousarray(bk_eff),
        relpad=relpad.astype(BF16), ident=ident, ident8=ident8,
        exprel=exprel.astype(BF16),
        id4pad=id4pad.astype(BF16), bdones=bdones.astype(BF16),
    ), c0


_BUILT = {}


def _get_nc(repeat: int = 1):
    if repeat not in _BUILT:
        _BUILT[repeat] = build_nc(repeat)
    return _BUILT[repeat]


def make_in_maps(inputs: dict, consts: dict) -> list:
    x = np.asarray(inputs["x"], dtype=np.float32).reshape(B * S, D).astype(BF16)
    in_maps = []
    for c in range(N_CORES):
        m = dict(consts)
        m["x"] = np.ascontiguousarray(x[c * TPC : (c + 1) * TPC])
        in_maps.append(m)
    return in_maps


def kernel(**inputs) -> np.ndarray:
    from concourse.bass_utils import run_bass_kernel_spmd

    consts, c0 = _host_constants(
        inputs["ln_g"], inputs["ln_b"], inputs["wq"], inputs["bq"],
        inputs["wk"], inputs["bk"], inputs["wv"], inputs["bv"],
        inputs["wo"], inputs["bo"], inputs["rel_bias"],
    )
    nc = _get_nc(1)
    in_maps = make_in_maps(inputs, consts)
    res = run_bass_kernel_spmd(nc, in_maps, core_ids=list(range(N_CORES)), trace=False)
    out = np.concatenate(
        [np.asarray(res.results[c]["y"], dtype=np.float32) for c in range(N_CORES)],
        axis=0,
    )
    out = out.reshape(B, S, D)
    if np.any(c0 != 0.0):
        out = out + c0.astype(np.float32)
    return out

